# revision 1
# baseline (speedup 1.0000x reference)
"""Distributed NT-Xent contrastive loss (heat-kernel similarity) on 8 TRN2 cores.

Math (reference semantics):
    h = concat(h_i, h_j)                               # [N, d], N=8192, d=256
    sim = exp(-(||x||^2 + ||y||^2 - 2 x.y) / 2)        # [N, N]
    per row r: loss_r = log(sum_{c != r} exp(sim[r,c])) - sim[r, partner(r)]
    loss = mean_r loss_r

Sharding: row-slabs of 1024 rows per core.  Each core's inputs are
column-ROLLED by its slab offset so the program is identical on every core
(pure SPMD, no core-id dependent addresses):
  - ht   [256, 8192] f32 : h^T rolled so the core's own slab occupies cols 0..1023
  - hrow [1024, 256] f32 : the core's slab rows (row-major, for row-norm bias)
  - eye  [128, 128] bf16 : identity mask for diagonal extraction
With this layout, for M-block m (128 rows), the self-diagonal sits at
cols m*128..m*128+128 and the positive-partner diagonal at 4096+m*128.. on
every core.

Device pipeline per M-block:
  PE   : q_raw = h_slab_blk @ h^T (bf16 ops, fp32 PSUM, K=2x128, N-tiles of 512)
  DVE  : q = (q_raw + (-|row|^2/2)) + (-|col|^2/2)   (scalar_tensor_tensor)
  ACT  : sim = Exp(q)                                 (1 call, 8192 free)
  ACT  : e2 = Exp(sim), S_r = row-sum via accum_out   (1 call, 8192 free)
  DVE  : pos_r, diag_r extracted via identity-mask multiply + accum_out
  final: loss_r = Ln(S_r - diag_r) - pos_r  -> out [128, 8] per core

Host: loss = sum(all cores' out) / N.
"""

import numpy as np
import ml_dtypes

import concourse.bass as bass
import concourse.bacc as bacc
import concourse.tile as tile
import concourse.mybir as mybir
from concourse.bass_utils import run_bass_kernel_spmd

BATCH = 4096
DIM = 256
N = 2 * BATCH            # 8192 total rows
NCORES = 8
SLAB = N // NCORES       # 1024 rows per core
MB = SLAB // 128         # 8 M-blocks of 128 rows
GROUP = 2048             # column group = 4 PSUM banks
NG = N // GROUP          # 4 groups
TILE = 512               # matmul free dim (1 PSUM bank)
NT = GROUP // TILE       # 4 col-tiles per group

FP32 = mybir.dt.float32
BF16 = mybir.dt.bfloat16


def _kernel_body(tc, ht, hrow, eye, out):
    nc = tc.nc
    A = mybir.AluOpType
    Act = mybir.ActivationFunctionType

    with (
        tc.tile_pool(name="singles", bufs=1) as singles,
        tc.tile_pool(name="chunks", bufs=2) as chunks,
        tc.tile_pool(name="sqchunks", bufs=2) as sqchunks,
        tc.tile_pool(name="qpool", bufs=2) as qpool,
        tc.tile_pool(name="simpool", bufs=2) as simpool,
        tc.tile_pool(name="e2pool", bufs=2) as e2pool,
        tc.tile_pool(name="small", bufs=2) as small,
        tc.tile_pool(name="hrpool", bufs=8) as hrpool,
        tc.tile_pool(name="psum", bufs=8, space="PSUM") as psum_pool,
    ):
        # ---- persistent tiles ----
        hTb0 = singles.tile([128, N], BF16, tag="hTb0")
        hTb1 = singles.tile([128, N], BF16, tag="hTb1")
        hTb = [hTb0, hTb1]
        sbc = singles.tile([128, N], FP32, tag="sbc")       # -|col|^2/2, bcast
        onesb = singles.tile([128, 128], BF16, tag="onesb")
        eye_s = singles.tile([128, 128], BF16, tag="eye_s")
        biasr = singles.tile([128, MB], FP32, tag="biasr")  # -|row|^2/2
        sqr = singles.tile([128, MB], FP32, tag="sqr")
        sv = singles.tile([128, MB], FP32, tag="sv")        # row-sums of exp(sim)
        e2dv = singles.tile([128, MB], FP32, tag="e2dv")    # exp(sim_diag)
        posv = singles.tile([128, MB], FP32, tag="posv")    # sim_pos

        nc.vector.memset(onesb, 1.0)
        nc.sync.dma_start(out=eye_s, in_=eye)

        # ---- row-norm bias from the slab in row-major layout ----
        for m in range(MB):
            hr = hrpool.tile([128, DIM], FP32, tag="hr")
            nc.gpsimd.dma_start(out=hr, in_=hrow[m * 128:(m + 1) * 128, :])
            scr = small.tile([128, DIM], FP32, tag="scr")
            nc.vector.scalar_tensor_tensor(
                scr, hr, 1.0, hr, A.mult, A.mult, accum_out=sqr[:, m:m + 1],
            )
        nc.vector.tensor_scalar_mul(biasr, sqr, -0.5)

        # ---- load h^T, cast to bf16, column norms via ones-matmul ----
        for g in range(NG):
            gs = slice(g * GROUP, (g + 1) * GROUP)
            sqcs = []
            for ki in range(2):
                hf = chunks.tile([128, GROUP], FP32, tag="hf")
                nc.sync.dma_start(out=hf, in_=ht[ki * 128:(ki + 1) * 128, gs])
                nc.vector.tensor_copy(out=hTb[ki][:, gs], in_=hf)
                sqc = sqchunks.tile([128, GROUP], BF16, tag=f"sqc{ki}")
                nc.vector.tensor_mul(sqc, hTb[ki][:, gs], hTb[ki][:, gs])
                sqcs.append(sqc)
            for t in range(NT):
                ts_ = slice(t * TILE, (t + 1) * TILE)
                ps = psum_pool.tile([128, TILE], FP32, tag="ps")
                for ki in range(2):
                    nc.tensor.matmul(
                        ps, onesb, sqcs[ki][:, ts_],
                        start=(ki == 0), stop=(ki == 1),
                    )
                nc.vector.tensor_scalar_mul(
                    sbc[:, g * GROUP + t * TILE:g * GROUP + (t + 1) * TILE],
                    ps, -0.5,
                )

        # ---- main loop over M-blocks ----
        for m in range(MB):
            ms = slice(m * 128, (m + 1) * 128)
            simb = simpool.tile([128, N], BF16, tag="simb")
            qg = qpool.tile([128, N], BF16, tag="qg")
            for g in range(NG):
                for t in range(NT):
                    c0 = g * GROUP + t * TILE
                    ps = psum_pool.tile([128, TILE], FP32, tag="ps")
                    for ki in range(2):
                        nc.tensor.matmul(
                            ps,
                            hTb[ki][:, ms],
                            hTb[ki][:, c0:c0 + TILE],
                            start=(ki == 0), stop=(ki == 1),
                        )
                    nc.vector.scalar_tensor_tensor(
                        qg[:, c0:c0 + TILE], ps, biasr[:, m:m + 1],
                        sbc[:, c0:c0 + TILE], A.add, A.add,
                    )
            nc.scalar.activation(simb, qg, Act.Exp)
            # positive-pair diagonal (cols 4096+m*128..), read before exp2
            pscr = small.tile([128, 128], BF16, tag="pscr")
            pc = BATCH + m * 128
            nc.vector.scalar_tensor_tensor(
                pscr, simb[:, pc:pc + 128], 1.0, eye_s, A.mult, A.mult,
                accum_out=posv[:, m:m + 1],
            )
            # exp(sim) with fused row-sum
            e2b = e2pool.tile([128, N], BF16, tag="e2b")
            nc.scalar.activation(e2b, simb, Act.Exp, accum_out=sv[:, m:m + 1])
            # self-diagonal of exp(sim) (cols m*128..)
            dscr = small.tile([128, 128], BF16, tag="dscr")
            nc.vector.scalar_tensor_tensor(
                dscr, e2b[:, ms], 1.0, eye_s, A.mult, A.mult,
                accum_out=e2dv[:, m:m + 1],
            )

        # ---- finalize: loss_r = Ln(S - exp(sim_diag)) - sim_pos ----
        t1 = singles.tile([128, MB], FP32, tag="t1")
        nc.vector.tensor_sub(t1, sv, e2dv)
        t2 = singles.tile([128, MB], FP32, tag="t2")
        nc.scalar.activation(t2, t1, Act.Ln)
        outv = singles.tile([128, MB], FP32, tag="outv")
        nc.vector.tensor_sub(outv, t2, posv)
        nc.sync.dma_start(out=out, in_=outv)


def build_bass():
    nc = bacc.Bacc("TRN2", target_bir_lowering=False, debug=False)
    ht = nc.dram_tensor("ht", [DIM, N], FP32, kind="ExternalInput").ap()
    hrow = nc.dram_tensor("hrow", [SLAB, DIM], FP32, kind="ExternalInput").ap()
    eye = nc.dram_tensor("eye", [128, 128], BF16, kind="ExternalInput").ap()
    out = nc.dram_tensor("out", [128, MB], FP32, kind="ExternalOutput").ap()
    with tile.TileContext(nc) as tc:
        _kernel_body(tc, ht, hrow, eye, out)
    nc.compile()
    return nc


def make_in_maps(h_i, h_j):
    h_i = np.asarray(h_i, dtype=np.float32)
    h_j = np.asarray(h_j, dtype=np.float32)
    h = np.concatenate([h_i, h_j], axis=0)          # [N, d]
    ht_full = np.ascontiguousarray(h.T)             # [d, N]
    eye = np.eye(128, dtype=ml_dtypes.bfloat16)
    in_maps = []
    for k in range(NCORES):
        ht_k = np.ascontiguousarray(np.roll(ht_full, -k * SLAB, axis=1))
        hrow_k = np.ascontiguousarray(h[k * SLAB:(k + 1) * SLAB, :])
        in_maps.append({"ht": ht_k, "hrow": hrow_k, "eye": eye})
    return in_maps


def reduce_outputs(results):
    total = 0.0
    for k in range(NCORES):
        total += np.asarray(results[k]["out"], dtype=np.float64).sum()
    return np.array(total / N, dtype=np.float32)


def kernel(h_i, h_j):
    nc = build_bass()
    in_maps = make_in_maps(h_i, h_j)
    res = run_bass_kernel_spmd(nc, in_maps, core_ids=list(range(NCORES)))
    return reduce_outputs(res.results)


if __name__ == "__main__":
    rng = np.random.default_rng(0)
    h_i = rng.standard_normal((BATCH, DIM), dtype=np.float32)
    h_j = rng.standard_normal((BATCH, DIM), dtype=np.float32)
    print("loss:", kernel(h_i, h_j))



# revision 3
# speedup vs baseline: 1.4611x; 1.4611x over previous
"""Distributed NT-Xent contrastive loss (heat-kernel similarity) on 8 TRN2 cores.

Math (reference semantics):
    h = concat(h_i, h_j)                               # [N, d], N=8192, d=256
    sim = exp(-(||x||^2 + ||y||^2 - 2 x.y) / 2)        # [N, N]
    per row r: loss_r = log(sum_{c != r} exp(sim[r,c])) - sim[r, partner(r)]
    loss = mean_r loss_r

Sharding: row-slabs of 1024 rows per core.  Each core's inputs are
column-ROLLED by its slab offset so the program is identical on every core
(pure SPMD, no core-id dependent addresses):
  - ht   [256, 8192] f32 : h^T rolled so the core's own slab occupies cols 0..1023
  - hrow [1024, 256] f32 : the core's slab rows (row-major, for row-norm bias)
  - eye  [128, 128] bf16 : identity mask for diagonal extraction
With this layout, for M-block m (128 rows), the self-diagonal sits at
cols m*128..m*128+128 and the positive-partner diagonal at 4096+m*128.. on
every core.

Device pipeline per M-block:
  PE   : q_raw = h_slab_blk @ h^T (bf16 ops, fp32 PSUM, K=2x128, N-tiles of 512)
  DVE  : q = (q_raw + (-|row|^2/2)) + (-|col|^2/2)   (scalar_tensor_tensor)
  ACT  : sim = Exp(q)                                 (1 call, 8192 free)
  ACT  : e2 = Exp(sim), S_r = row-sum via accum_out   (1 call, 8192 free)
  DVE  : pos_r, diag_r extracted via identity-mask multiply + accum_out
  final: loss_r = Ln(S_r - diag_r) - pos_r  -> out [128, 8] per core

Host: loss = sum(all cores' out) / N.
"""

import numpy as np
import ml_dtypes

import concourse.bass as bass
import concourse.bacc as bacc
import concourse.tile as tile
import concourse.mybir as mybir
from concourse.bass_utils import run_bass_kernel_spmd

BATCH = 4096
DIM = 256
N = 2 * BATCH            # 8192 total rows
NCORES = 8
SLAB = N // NCORES       # 1024 rows per core
MB = SLAB // 128         # 8 M-blocks of 128 rows
GROUP = 2048             # column group = 4 PSUM banks
NG = N // GROUP          # 4 groups
TILE = 512               # matmul free dim (1 PSUM bank)
NT = GROUP // TILE       # 4 col-tiles per group

FP32 = mybir.dt.float32
BF16 = mybir.dt.bfloat16


def _kernel_body(tc, ht, hrow, eye, out):
    nc = tc.nc
    A = mybir.AluOpType
    Act = mybir.ActivationFunctionType

    with (
        tc.tile_pool(name="singles", bufs=1) as singles,
        tc.tile_pool(name="chunks", bufs=2) as chunks,
        tc.tile_pool(name="sqchunks", bufs=2) as sqchunks,
        tc.tile_pool(name="qpool", bufs=2) as qpool,
        tc.tile_pool(name="simpool", bufs=2) as simpool,
        tc.tile_pool(name="e2pool", bufs=2) as e2pool,
        tc.tile_pool(name="small", bufs=2) as small,
        tc.tile_pool(name="hrpool", bufs=8) as hrpool,
        tc.tile_pool(name="psum", bufs=8, space="PSUM") as psum_pool,
    ):
        # ---- persistent tiles ----
        hTb0 = singles.tile([128, N], BF16, tag="hTb0")
        hTb1 = singles.tile([128, N], BF16, tag="hTb1")
        hTb = [hTb0, hTb1]
        sbc = singles.tile([128, N], FP32, tag="sbc")       # -|col|^2/2, bcast
        onesb = singles.tile([128, 128], BF16, tag="onesb")
        eye_s = singles.tile([128, 128], BF16, tag="eye_s")
        biasr = singles.tile([128, MB], FP32, tag="biasr")  # -|row|^2/2
        sqr = singles.tile([128, MB], FP32, tag="sqr")
        sv = singles.tile([128, MB], FP32, tag="sv")        # row-sums of exp(sim)
        e2dv = singles.tile([128, MB], FP32, tag="e2dv")    # exp(sim_diag)
        posv = singles.tile([128, MB], FP32, tag="posv")    # sim_pos

        nc.vector.memset(onesb, 1.0)
        nc.sync.dma_start(out=eye_s, in_=eye)

        # ---- row-norm bias from the slab in row-major layout ----
        for m in range(MB):
            hr = hrpool.tile([128, DIM], FP32, tag="hr")
            nc.gpsimd.dma_start(out=hr, in_=hrow[m * 128:(m + 1) * 128, :])
            scr = small.tile([128, DIM], FP32, tag="scr")
            nc.vector.scalar_tensor_tensor(
                scr, hr, 1.0, hr, A.mult, A.mult, accum_out=sqr[:, m:m + 1],
            )
        nc.vector.tensor_scalar_mul(biasr, sqr, -0.5)

        # ---- load h^T, cast to bf16, column norms via ones-matmul ----
        for g in range(NG):
            gs = slice(g * GROUP, (g + 1) * GROUP)
            sqcs = []
            for ki in range(2):
                hf = chunks.tile([128, GROUP], FP32, tag="hf")
                nc.sync.dma_start(out=hf, in_=ht[ki * 128:(ki + 1) * 128, gs])
                nc.vector.tensor_copy(out=hTb[ki][:, gs], in_=hf)
                sqc = sqchunks.tile([128, GROUP], BF16, tag=f"sqc{ki}")
                nc.vector.tensor_mul(sqc, hTb[ki][:, gs], hTb[ki][:, gs])
                sqcs.append(sqc)
            for t in range(NT):
                ts_ = slice(t * TILE, (t + 1) * TILE)
                ps = psum_pool.tile([128, TILE], FP32, tag="ps")
                for ki in range(2):
                    nc.tensor.matmul(
                        ps, onesb, sqcs[ki][:, ts_],
                        start=(ki == 0), stop=(ki == 1),
                    )
                nc.vector.tensor_scalar_mul(
                    sbc[:, g * GROUP + t * TILE:g * GROUP + (t + 1) * TILE],
                    ps, -0.5,
                )

        # ---- main loop over M-blocks ----
        for m in range(MB):
            ms = slice(m * 128, (m + 1) * 128)
            simb = simpool.tile([128, N], BF16, tag="simb")
            qg = qpool.tile([128, N], BF16, tag="qg")
            for g in range(NG):
                for t in range(NT):
                    c0 = g * GROUP + t * TILE
                    ps = psum_pool.tile([128, TILE], FP32, tag="ps")
                    for ki in range(2):
                        nc.tensor.matmul(
                            ps,
                            hTb[ki][:, ms],
                            hTb[ki][:, c0:c0 + TILE],
                            start=(ki == 0), stop=(ki == 1),
                        )
                    nc.vector.scalar_tensor_tensor(
                        qg[:, c0:c0 + TILE], ps, biasr[:, m:m + 1],
                        sbc[:, c0:c0 + TILE], A.add, A.add,
                    )
            nc.scalar.activation(simb, qg, Act.Exp)
            # positive-pair diagonal (cols 4096+m*128..), read before exp2
            pscr = small.tile([128, 128], BF16, tag="pscr")
            pc = BATCH + m * 128
            nc.vector.scalar_tensor_tensor(
                pscr, simb[:, pc:pc + 128], 1.0, eye_s, A.mult, A.mult,
                accum_out=posv[:, m:m + 1],
            )
            # exp(sim) with fused row-sum
            e2b = e2pool.tile([128, N], BF16, tag="e2b")
            nc.scalar.activation(e2b, simb, Act.Exp, accum_out=sv[:, m:m + 1])
            # self-diagonal of exp(sim) (cols m*128..)
            dscr = small.tile([128, 128], BF16, tag="dscr")
            nc.vector.scalar_tensor_tensor(
                dscr, e2b[:, ms], 1.0, eye_s, A.mult, A.mult,
                accum_out=e2dv[:, m:m + 1],
            )

        # ---- finalize: loss_r = Ln(S - exp(sim_diag)) - sim_pos ----
        t1 = singles.tile([128, MB], FP32, tag="t1")
        nc.vector.tensor_sub(t1, sv, e2dv)
        t2 = singles.tile([128, MB], FP32, tag="t2")
        nc.scalar.activation(t2, t1, Act.Ln)
        outv = singles.tile([128, MB], FP32, tag="outv")
        nc.vector.tensor_sub(outv, t2, posv)
        nc.sync.dma_start(out=out, in_=outv)


def build_bass():
    nc = bacc.Bacc("TRN2", target_bir_lowering=False, debug=False)
    ht = nc.dram_tensor("ht", [DIM, N], FP32, kind="ExternalInput").ap()
    hrow = nc.dram_tensor("hrow", [SLAB, DIM], FP32, kind="ExternalInput").ap()
    eye = nc.dram_tensor("eye", [128, 128], BF16, kind="ExternalInput").ap()
    out = nc.dram_tensor("out", [128, MB], FP32, kind="ExternalOutput").ap()
    with tile.TileContext(nc) as tc:
        _kernel_body(tc, ht, hrow, eye, out)
    nc.compile()
    return nc


def make_in_maps(h_i, h_j):
    h_i = np.asarray(h_i, dtype=np.float32)
    h_j = np.asarray(h_j, dtype=np.float32)
    h = np.concatenate([h_i, h_j], axis=0)          # [N, d]
    ht_full = np.ascontiguousarray(h.T)             # [d, N]
    eye = np.eye(128, dtype=ml_dtypes.bfloat16)
    in_maps = []
    for k in range(NCORES):
        ht_k = np.ascontiguousarray(np.roll(ht_full, -k * SLAB, axis=1))
        hrow_k = np.ascontiguousarray(h[k * SLAB:(k + 1) * SLAB, :])
        in_maps.append({"ht": ht_k, "hrow": hrow_k, "eye": eye})
    return in_maps


def reduce_outputs(results):
    total = 0.0
    for k in range(NCORES):
        total += np.asarray(results[k]["out"], dtype=np.float64).sum()
    return np.array(total / N, dtype=np.float32)


def kernel(h_i, h_j):
    nc = build_bass()
    in_maps = make_in_maps(h_i, h_j)
    res = run_bass_kernel_spmd(nc, in_maps, core_ids=list(range(NCORES)))
    return reduce_outputs(res.results)


if __name__ == "__main__":
    rng = np.random.default_rng(0)
    h_i = rng.standard_normal((BATCH, DIM), dtype=np.float32)
    h_j = rng.standard_normal((BATCH, DIM), dtype=np.float32)
    print("loss:", kernel(h_i, h_j))

